# revision 31
# baseline (speedup 1.0000x reference)
"""Trainium2 Bass kernel for CRF mean-field iteration (nn_CRF).

Math (derived from the reference):
    comp = -I  =>  each iteration is   x <- x0 + w * smooth(softmax(x, C))
    output = log_softmax(x_final, C)
where smooth = per-channel separable 11-tap Gaussian blur over H then W
('same' zero padding, center tap zeroed, per-sample spacing).

Strategy (per core, 2 samples, pure data parallel over batch). The
correctness gate is rel_err < 2e-2, so the whole p/conv path runs in bf16
(measured ~2.8e-3 end-to-end on HW); PSUM accumulates f32.

Per iteration (its 0..3), per channel:
  - p = e*r (flat bf16 DVE mul, 2x packed mode), deferred from the previous
    iteration's softmax so the DVE work rides under the PE conv stream.
  - H-conv as matmul with the data stationary: out1[w,h'] = sum_h p[h,w]
    Th[h,h'] (banded Toeplitz moving operand, built on host) -> one 3-bank
    PSUM tile, drained to bf16 o1 by the DVE (ScalarE only does exps, so
    the drain never queues behind ACT work).
  - W-conv per w-chunk into its own single PSUM bank (4 rotating banks):
    the bank is seeded with x0 by an identity-stationary matmul, the
    banded Tw matmuls accumulate on top (PSUM ends holding x = x0 + s),
    and ScalarE exp consumes the bank the moment its group stops.
  - channel-sum tree: the 4-channel-group pair adds run on the otherwise
    idle GpSimd; partial sums accumulate incrementally so at the last
    channel only S = tpre + e15, 1/S and the bf16 cast remain, emitted
    per j-chunk to shorten the iteration boundary.
The final log_softmax's exps/tree are folded INTO the last iteration's
channel loop (ScalarE is otherwise idle there), x0 lives in four
per-channel-group tiles so the next sample's DMA starts mid-loop, and the
final per-channel subtract+output-DMA is spliced into the next sample's
first conv pass.
"""

import sys

if "/opt/trn_rl_repo" not in sys.path:
    sys.path.insert(0, "/opt/trn_rl_repo")

from contextlib import ExitStack

import numpy as np

import concourse.bass as bass
import concourse.tile as tile
from concourse import bacc, mybir

F32 = mybir.dt.float32
BF16 = mybir.dt.bfloat16
AF = mybir.ActivationFunctionType

B, C, H, W = 16, 16, 384, 384
N_CORES = 8
BPC = B // N_CORES  # samples per core
N_ITER = 5
FS = 11
HALF = FS // 2  # 5
P = 128
NCH = H // P  # 3 h-chunks
NCW = W // P  # 3 w-chunks
NW = NCH * W  # flattened (h-chunk, w) free size


def _band(j, n):
    """Output-column range touched by contraction chunk j of a banded T."""
    return max(0, P * j - HALF), min(n, P * j + P + HALF)


def _f2(ap):
    return ap.rearrange("p a b -> p (a b)")


def _f3(ap):
    return ap.rearrange("p a b c -> p (a b c)")


def _crf_kernel(ctx, tc, out_d, x_in, th_in, tw_in, id_in, n_samples, n_iter, full_j0):
    nc = tc.nc

    state = ctx.enter_context(tc.tile_pool(name="state", bufs=1))
    mats = ctx.enter_context(tc.tile_pool(name="mats", bufs=1))
    stage = ctx.enter_context(tc.tile_pool(name="stage", bufs=3))
    smax = ctx.enter_context(tc.tile_pool(name="smax", bufs=1))
    small = ctx.enter_context(tc.tile_pool(name="small", bufs=2))
    psA = ctx.enter_context(tc.tile_pool(name="psA", bufs=4, space="PSUM"))
    psB = ctx.enter_context(tc.tile_pool(name="psB", bufs=4, space="PSUM"))

    xbuf = state.tile([P, C, NCH, W], F32, tag="xbuf")
    x0g = [
        state.tile([P, 4, NCH, W], BF16, tag=f"x0g{g}", name=f"x0g{g}")
        for g in range(4)
    ]
    ebuf = state.tile([P, C, NCH, W], BF16, tag="ebuf")
    ident = state.tile([P, P], BF16, tag="ident")
    nc.sync.dma_start(out=ident[:], in_=id_in[:])

    def x0c(c):
        return x0g[c // 4][:, c % 4]

    def new_tree():
        st = {}
        for t in ("v0", "v1", "v2", "u3", "tacc", "rb"):
            st[t] = smax.tile([P, NW], BF16, tag=t, name=t)
        st["S"] = smax.tile([P, NW], F32, tag="S", name="S")
        st["r"] = smax.tile([P, NW], F32, tag="r", name="r")
        return st

    def tree_step(st, c, with_recip=True, u_eng=None):
        """Incremental channel-sum; call right after exp(c) is emitted.
        tacc accumulates v0+v1, then +v2, then +u3 in place. u_eng picks
        the engine for the wide pair-add: GpSimd is fine mid-loop (latency
        hidden under convs) but poisons latency-critical chains."""
        V = nc.vector
        if c == 3 or c == 7 or c == 11:
            g = c // 4
            ut = small.tile([P, 2 * NW], BF16, tag="tu")
            (u_eng or nc.vector).tensor_add(
                ut[:], _f3(ebuf[:, 4 * g : 4 * g + 2]),
                _f3(ebuf[:, 4 * g + 2 : 4 * g + 4]),
            )
            V.tensor_add(st[f"v{g}"][:], ut[:, 0:NW], ut[:, NW : 2 * NW])
            if c == 7:
                V.tensor_add(st["tacc"][:], st["v0"][:], st["v1"][:])
            elif c == 11:
                V.tensor_add(st["tacc"][:], st["tacc"][:], st["v2"][:])
        elif c == 13:
            V.tensor_add(st["u3"][:], _f2(ebuf[:, 12]), _f2(ebuf[:, 13]))
        elif c == 14:
            V.tensor_add(st["u3"][:], st["u3"][:], _f2(ebuf[:, 14]))
            V.tensor_add(st["tacc"][:], st["tacc"][:], st["u3"][:])
        elif c == 15:
            # Tail, j-split: only S = tacc + e15, 1/S, bf16 cast remain.
            for j in range(NCH):
                sl = slice(j * W, (j + 1) * W)
                V.tensor_add(st["S"][:, sl], st["tacc"][:, sl], ebuf[:, 15, j])
                if with_recip:
                    V.reciprocal_approx_fast(st["r"][:, sl], st["S"][:, sl])
                    V.tensor_scalar_mul(st["rb"][:, sl], st["r"][:, sl], 1.0)

    def load_sample(b):
        for cg in range(4):
            nc.sync.dma_start(
                out=x0g[cg][:],
                in_=x_in[b, 4 * cg : 4 * cg + 4].rearrange(
                    "c (j p) w -> p c j w", p=P
                ),
            )
        th_sb = mats.tile([P, NCH, H], BF16, tag="th")
        tw_sb = mats.tile([P, NCW, W], BF16, tag="tw")
        nc.sync.dma_start(out=th_sb[:], in_=th_in[b].rearrange("(j p) n -> p j n", p=P))
        nc.sync.dma_start(out=tw_sb[:], in_=tw_in[b].rearrange("(j p) n -> p j n", p=P))
        return th_sb, tw_sb

    def emit_exp_cg(src4, cg):
        nc.scalar.activation(
            out=_f3(ebuf[:, 4 * cg : 4 * cg + 4]), in_=_f3(src4), func=AF.Exp
        )

    def emit_prologue(st):
        for cg in range(4):
            emit_exp_cg(x0g[cg][:], cg)
            if cg < 3:
                tree_step(st, 4 * cg + 3)
            else:
                tree_step(st, 13)
                tree_step(st, 14)
                tree_step(st, 15)

    def conv_iters(b, th_sb, tw_sb, st, pending):
        def emit_pmul(c):
            # deferred p = e * r (j-split for the first channels so the
            # boundary chain only gates 1/3 of the first conv)
            if c < 2:
                for j in range(NCH):
                    nc.vector.tensor_mul(
                        out=ebuf[:, c, j], in0=ebuf[:, c, j],
                        in1=st["rb"][:, j * W : (j + 1) * W],
                    )
            else:
                nc.vector.tensor_mul(
                    out=_f2(ebuf[:, c]), in0=_f2(ebuf[:, c]), in1=st["rb"][:]
                )

        def emit_H(c, last):
            # H-conv into per-m single-bank PSUM tiles, each drained to
            # its o1 slice right after its accumulation group stops —
            # the 3-deep bank rotation means the PE never waits on a
            # drain.  Drains split: m0 -> ScalarE, m1/m2 -> DVE (the
            # boundary channel and the last iteration go all-Scalar,
            # where the DVE runs the softmax tail / x-updates).
            o1 = stage.tile([P, NCW, H], BF16, tag="o1")
            for m in range(NCW):
                pAm = psA.tile([P, 512], F32, tag="pa")
                for j in range(NCH):
                    # CoreSim needs j==0 to cover the full width (its
                    # pending-zero model can't mix accumulate/overwrite
                    # in one matmul); HW has_written handles the banded
                    # overlap per element.
                    n0, n1 = (0, H) if (j == 0 and full_j0) else _band(j, H)
                    nc.tensor.matmul(
                        pAm[:, n0:n1],
                        lhsT=ebuf[:, c, j, m * P : (m + 1) * P],
                        rhs=th_sb[:, j, n0:n1],
                        start=(j == 0),
                        stop=(j == NCH - 1),
                    )
                if last or c == 15 or m == 0:
                    nc.scalar.copy(out=o1[:, m], in_=pAm[:, 0:H])
                else:
                    nc.vector.tensor_scalar_mul(o1[:, m], pAm[:, 0:H], 1.0)
            return o1

        for it in range(n_iter):
            last = it == n_iter - 1
            nst = new_tree()  # on the last iteration this is the final-pass tree
            # Software pipelining: channel c+1's H-conv is emitted BEFORE
            # channel c's W-conv, so the in-order PE always has a full
            # H-conv of independent work queued while W(c) waits on the o1
            # drain of channel c.
            emit_pmul(0)
            o1 = emit_H(0, last)
            for c in range(C):
                if c + 1 < C:
                    emit_pmul(c + 1)
                    o1_nxt = emit_H(c + 1, last)
                for m in range(NCH):
                    pBm = psB.tile([P, 512], F32, tag="pb")
                    if not last:
                        # Seed the bank with x0 (identity matmul); the W-conv
                        # accumulates on top so it ends holding x = x0 + s.
                        nc.tensor.matmul(
                            pBm[:, 0:W],
                            lhsT=ident[:],
                            rhs=x0c(c)[:, m],
                            start=True,
                            stop=False,
                        )
                    for j in range(NCW):
                        n0, n1 = (0, W) if (j == 0 and full_j0 and last) else _band(j, W)
                        nc.tensor.matmul(
                            pBm[:, n0:n1],
                            lhsT=o1[:, j, m * P : (m + 1) * P],
                            rhs=tw_sb[:, j, n0:n1],
                            start=(j == 0 and last),
                            stop=(j == NCW - 1),
                        )
                    if not last:
                        # e = exp(x) straight out of the bank.
                        nc.scalar.activation(
                            out=ebuf[:, c, m], in_=pBm[:, 0:W], func=AF.Exp
                        )
                    else:
                        nc.vector.tensor_add(
                            out=xbuf[:, c, m], in0=x0c(c)[:, m], in1=pBm[:, 0:W]
                        )
                o1 = o1_nxt if c + 1 < C else None
                if not last:
                    tree_step(nst, c, u_eng=nc.gpsimd if c in (3, 7) else nc.vector)
                else:
                    # fold the final log_softmax's exps/tree into this loop;
                    # ScalarE has no per-channel exps here.
                    if c in (3, 7, 11):
                        emit_exp_cg(xbuf[:, c - 3 : c + 1], c // 4)
                        tree_step(nst, c, with_recip=False, u_eng=nc.gpsimd)
                    elif c == 15:
                        emit_exp_cg(xbuf[:, 12:16], 3)
                        tree_step(nst, 13, with_recip=False)
                        tree_step(nst, 14, with_recip=False)
                        tree_step(nst, 15, with_recip=False)
                # splice in the previous sample's final subtract+DMA work
                for _ in range(2):
                    if pending:
                        pending.pop(0)()
            st = nst
        return st  # the final-pass tree (holds S of log-softmax)

    def emit_final_tail(b, fst):
        """Ln + per-channel subtract/DMA closures for sample b."""
        # Dedicated tag: lball stays live while the next sample's softmax
        # reciprocal (tag "r") is being written.
        lball = smax.tile([P, NW], F32, tag="lb", name="lb")
        nc.scalar.activation(out=lball[:], in_=fst["S"][:], func=AF.Ln)
        lb_v = lball[:].rearrange("p (a b) -> p a b", a=NCH)
        pending = []
        for c in range(C):
            def mk(c=c):
                def go():
                    sout = stage.tile([P, NCH, W], F32, tag="sout")
                    nc.vector.tensor_sub(out=sout[:], in0=xbuf[:, c], in1=lb_v)
                    nc.sync.dma_start(
                        out=out_d[b, c].rearrange("(j p) w -> p j w", p=P),
                        in_=sout[:],
                    )
                return go
            pending.append(mk())
        return pending

    pending = []
    for b in range(n_samples):
        th_sb, tw_sb = load_sample(b)
        st0 = new_tree()
        emit_prologue(st0)
        fst = conv_iters(b, th_sb, tw_sb, st0, pending)
        pending = emit_final_tail(b, fst)
    for fn in pending:
        fn()


def build_nc(n_samples=BPC, n_iter=N_ITER, full_j0=False):
    # Bacc (not plain Bass): its compile() pass legalizes multi-wait
    # instructions via InstEventSemaphore — walrus caps regular instructions
    # at ONE sync wait.
    nc = bacc.Bacc()
    x_in = nc.dram_tensor("x", [n_samples, C, H, W], BF16, kind="ExternalInput")
    th_in = nc.dram_tensor("th", [n_samples, H, H], BF16, kind="ExternalInput")
    tw_in = nc.dram_tensor("tw", [n_samples, W, W], BF16, kind="ExternalInput")
    id_in = nc.dram_tensor("ident", [P, P], BF16, kind="ExternalInput")
    out_d = nc.dram_tensor("out", [n_samples, C, H, W], F32, kind="ExternalOutput")
    with tile.TileContext(nc) as tc:
        with ExitStack() as ctx:
            _crf_kernel(
                ctx, tc, out_d, x_in, th_in, tw_in, id_in, n_samples, n_iter, full_j0
            )
    nc.finalize()
    return nc


def make_toeplitz(spacing, inv_theta, size, weight=1.0):
    """Banded symmetric Toeplitz matrix for the 1D 'same' correlation."""
    d = spacing * np.arange(-(FS // 2), FS // 2 + 1, dtype=np.float32)
    k = np.exp(-((d * inv_theta) ** 2) / 2.0).astype(np.float32)
    k[FS // 2] = 0.0
    t = np.zeros((size, size), dtype=np.float32)
    for tap in range(FS):
        off = tap - FS // 2  # out[h] += k[tap] * x[h + off]
        idx = np.arange(max(0, -off), min(size, size - off))
        t[idx + off, idx] = k[tap]
    return (t * weight).astype(np.float32)


def host_prep(x, spatial_spacings, smoothness_weight, inv_smoothness_theta):
    """Build per-sample Th (H-conv) and weight-scaled Tw (W-conv) matrices
    plus the bf16 copy of x; all conv-path operands ship as bf16."""
    import ml_dtypes

    w = float(np.asarray(smoothness_weight))
    th = np.stack(
        [
            make_toeplitz(float(spatial_spacings[b, 0]), float(inv_smoothness_theta[0]), H)
            for b in range(x.shape[0])
        ]
    ).astype(ml_dtypes.bfloat16)
    tw = np.stack(
        [
            make_toeplitz(
                float(spatial_spacings[b, 1]), float(inv_smoothness_theta[1]), W, weight=w
            )
            for b in range(x.shape[0])
        ]
    ).astype(ml_dtypes.bfloat16)
    xb = np.ascontiguousarray(x).astype(ml_dtypes.bfloat16)
    return xb, th, tw


def make_ident():
    import ml_dtypes

    return np.eye(P, dtype=np.float32).astype(ml_dtypes.bfloat16)


_NC_CACHE = {}


def kernel(x, spatial_spacings, smoothness_weight, inv_smoothness_theta):
    from concourse.bass_utils import run_bass_kernel_spmd

    x = np.ascontiguousarray(np.asarray(x), dtype=np.float32)
    spatial_spacings = np.asarray(spatial_spacings, dtype=np.float32)
    xb, th, tw = host_prep(x, spatial_spacings, smoothness_weight, inv_smoothness_theta)
    ident = make_ident()

    key = (BPC, N_ITER)
    if key not in _NC_CACHE:
        _NC_CACHE[key] = build_nc(BPC, N_ITER)
    nc = _NC_CACHE[key]

    core_ids = list(range(N_CORES))
    in_maps = []
    for i in core_ids:
        sl = slice(i * BPC, (i + 1) * BPC)
        in_maps.append({"x": xb[sl], "th": th[sl], "tw": tw[sl], "ident": ident})
    res = run_bass_kernel_spmd(nc, in_maps, core_ids)
    out = np.concatenate([res.results[i]["out"] for i in core_ids], axis=0)
    return out.astype(np.float32)


if __name__ == "__main__":
    rng = np.random.default_rng(0)
    x = rng.standard_normal((B, C, H, W), dtype=np.float32)
    out = kernel(
        x,
        np.ones((B, 2), np.float32),
        np.float32(1.0),
        np.ones((2,), np.float32),
    )
    print(out.shape, out.dtype)


# revision 33
# speedup vs baseline: 1.0017x; 1.0017x over previous
"""Trainium2 Bass kernel for CRF mean-field iteration (nn_CRF).

Math (derived from the reference):
    comp = -I  =>  each iteration is   x <- x0 + w * smooth(softmax(x, C))
    output = log_softmax(x_final, C)
where smooth = per-channel separable 11-tap Gaussian blur over H then W
('same' zero padding, center tap zeroed, per-sample spacing).

Strategy (per core, 2 samples, pure data parallel over batch). The
correctness gate is rel_err < 2e-2, so the whole p/conv path runs in bf16
(measured ~2.8e-3 end-to-end on HW); PSUM accumulates f32.

Per iteration (its 0..3), per channel:
  - p = e*r (flat bf16 DVE mul, 2x packed mode), deferred from the previous
    iteration's softmax so the DVE work rides under the PE conv stream.
  - H-conv as matmul with the data stationary: out1[w,h'] = sum_h p[h,w]
    Th[h,h'] (banded Toeplitz moving operand, built on host) -> one 3-bank
    PSUM tile, drained to bf16 o1 by the DVE (ScalarE only does exps, so
    the drain never queues behind ACT work).
  - W-conv per w-chunk into its own single PSUM bank (4 rotating banks):
    the bank is seeded with x0 by an identity-stationary matmul, the
    banded Tw matmuls accumulate on top (PSUM ends holding x = x0 + s),
    and ScalarE exp consumes the bank the moment its group stops.
  - channel-sum tree: the 4-channel-group pair adds run on the otherwise
    idle GpSimd; partial sums accumulate incrementally so at the last
    channel only S = tpre + e15, 1/S and the bf16 cast remain, emitted
    per j-chunk to shorten the iteration boundary.
The final log_softmax's exps/tree are folded INTO the last iteration's
channel loop (ScalarE is otherwise idle there), x0 lives in four
per-channel-group tiles so the next sample's DMA starts mid-loop, and the
final per-channel subtract+output-DMA is spliced into the next sample's
first conv pass.
"""

import sys

if "/opt/trn_rl_repo" not in sys.path:
    sys.path.insert(0, "/opt/trn_rl_repo")

from contextlib import ExitStack

import numpy as np

import concourse.bass as bass
import concourse.tile as tile
from concourse import bacc, mybir

F32 = mybir.dt.float32
BF16 = mybir.dt.bfloat16
AF = mybir.ActivationFunctionType

B, C, H, W = 16, 16, 384, 384
N_CORES = 8
BPC = B // N_CORES  # samples per core
N_ITER = 5
FS = 11
HALF = FS // 2  # 5
P = 128
NCH = H // P  # 3 h-chunks
NCW = W // P  # 3 w-chunks
NW = NCH * W  # flattened (h-chunk, w) free size


def _band(j, n):
    """Output-column range touched by contraction chunk j of a banded T."""
    return max(0, P * j - HALF), min(n, P * j + P + HALF)


def _f2(ap):
    return ap.rearrange("p a b -> p (a b)")


def _f3(ap):
    return ap.rearrange("p a b c -> p (a b c)")


def _crf_kernel(ctx, tc, out_d, x_in, th_in, tw_in, id_in, n_samples, n_iter, full_j0):
    nc = tc.nc

    state = ctx.enter_context(tc.tile_pool(name="state", bufs=1))
    mats = ctx.enter_context(tc.tile_pool(name="mats", bufs=2))
    stage = ctx.enter_context(tc.tile_pool(name="stage", bufs=2))
    smax = ctx.enter_context(tc.tile_pool(name="smax", bufs=1))
    small = ctx.enter_context(tc.tile_pool(name="small", bufs=2))
    psA = ctx.enter_context(tc.tile_pool(name="psA", bufs=3, space="PSUM"))
    psB = ctx.enter_context(tc.tile_pool(name="psB", bufs=5, space="PSUM"))

    xbuf = state.tile([P, C, NCH, W], F32, tag="xbuf")
    x0g = [
        state.tile([P, 4, NCH, W], BF16, tag=f"x0g{g}", name=f"x0g{g}")
        for g in range(4)
    ]
    ebuf = state.tile([P, C, NCH, W], BF16, tag="ebuf")
    ident = state.tile([P, P], BF16, tag="ident")
    nc.sync.dma_start(out=ident[:], in_=id_in[:])

    def x0c(c):
        return x0g[c // 4][:, c % 4]

    def new_tree():
        st = {}
        for t in ("v0", "v1", "v2", "u3", "tacc", "rb"):
            st[t] = smax.tile([P, NW], BF16, tag=t, name=t)
        st["S"] = smax.tile([P, NW], F32, tag="S", name="S")
        st["r"] = smax.tile([P, NW], F32, tag="r", name="r")
        return st

    def tree_step(st, c, with_recip=True, u_eng=None):
        """Incremental channel-sum; call right after exp(c) is emitted.
        tacc accumulates v0+v1, then +v2, then +u3 in place. u_eng picks
        the engine for the wide pair-add: GpSimd is fine mid-loop (latency
        hidden under convs) but poisons latency-critical chains."""
        V = nc.vector
        if c == 3 or c == 7 or c == 11:
            g = c // 4
            ut = small.tile([P, 2 * NW], BF16, tag="tu")
            (u_eng or nc.vector).tensor_add(
                ut[:], _f3(ebuf[:, 4 * g : 4 * g + 2]),
                _f3(ebuf[:, 4 * g + 2 : 4 * g + 4]),
            )
            V.tensor_add(st[f"v{g}"][:], ut[:, 0:NW], ut[:, NW : 2 * NW])
            if c == 7:
                V.tensor_add(st["tacc"][:], st["v0"][:], st["v1"][:])
            elif c == 11:
                V.tensor_add(st["tacc"][:], st["tacc"][:], st["v2"][:])
        elif c == 13:
            V.tensor_add(st["u3"][:], _f2(ebuf[:, 12]), _f2(ebuf[:, 13]))
        elif c == 14:
            V.tensor_add(st["u3"][:], st["u3"][:], _f2(ebuf[:, 14]))
            V.tensor_add(st["tacc"][:], st["tacc"][:], st["u3"][:])
        elif c == 15:
            # Tail, j-split: only S = tacc + e15, 1/S, bf16 cast remain.
            for j in range(NCH):
                sl = slice(j * W, (j + 1) * W)
                V.tensor_add(st["S"][:, sl], st["tacc"][:, sl], ebuf[:, 15, j])
                if with_recip:
                    V.reciprocal_approx_fast(st["r"][:, sl], st["S"][:, sl])
                    V.tensor_scalar_mul(st["rb"][:, sl], st["r"][:, sl], 1.0)

    def load_sample(b):
        for cg in range(4):
            nc.sync.dma_start(
                out=x0g[cg][:],
                in_=x_in[b, 4 * cg : 4 * cg + 4].rearrange(
                    "c (j p) w -> p c j w", p=P
                ),
            )
        th_sb = mats.tile([P, NCH, H], BF16, tag="th")
        tw_sb = mats.tile([P, NCW, W], BF16, tag="tw")
        nc.sync.dma_start(out=th_sb[:], in_=th_in[b].rearrange("(j p) n -> p j n", p=P))
        nc.sync.dma_start(out=tw_sb[:], in_=tw_in[b].rearrange("(j p) n -> p j n", p=P))
        return th_sb, tw_sb

    def emit_exp_cg(src4, cg):
        nc.scalar.activation(
            out=_f3(ebuf[:, 4 * cg : 4 * cg + 4]), in_=_f3(src4), func=AF.Exp
        )

    def emit_prologue(st):
        for cg in range(4):
            emit_exp_cg(x0g[cg][:], cg)
            if cg < 3:
                tree_step(st, 4 * cg + 3)
            else:
                tree_step(st, 13)
                tree_step(st, 14)
                tree_step(st, 15)

    def conv_iters(b, th_sb, tw_sb, st, pending):
        def emit_pmul(c):
            # deferred p = e * r (j-split for the first channels so the
            # boundary chain only gates 1/3 of the first conv)
            if c < 2:
                for j in range(NCH):
                    nc.vector.tensor_mul(
                        out=ebuf[:, c, j], in0=ebuf[:, c, j],
                        in1=st["rb"][:, j * W : (j + 1) * W],
                    )
            else:
                nc.vector.tensor_mul(
                    out=_f2(ebuf[:, c]), in0=_f2(ebuf[:, c]), in1=st["rb"][:]
                )

        def emit_H(c, last):
            # H-conv into per-m single-bank PSUM tiles, each drained to
            # its o1 slice right after its accumulation group stops —
            # the 3-deep bank rotation means the PE never waits on a
            # drain.  Drains split: m0 -> ScalarE, m1/m2 -> DVE (the
            # boundary channel and the last iteration go all-Scalar,
            # where the DVE runs the softmax tail / x-updates).
            o1 = stage.tile([P, NCW, H], BF16, tag="o1")
            for m in range(NCW):
                pAm = psA.tile([P, 512], F32, tag="pa")
                for j in range(NCH):
                    # CoreSim needs j==0 to cover the full width (its
                    # pending-zero model can't mix accumulate/overwrite
                    # in one matmul); HW has_written handles the banded
                    # overlap per element.
                    n0, n1 = (0, H) if (j == 0 and full_j0) else _band(j, H)
                    nc.tensor.matmul(
                        pAm[:, n0:n1],
                        lhsT=ebuf[:, c, j, m * P : (m + 1) * P],
                        rhs=th_sb[:, j, n0:n1],
                        start=(j == 0),
                        stop=(j == NCH - 1),
                    )
                if last or c == 15 or m == 0:
                    nc.scalar.copy(out=o1[:, m], in_=pAm[:, 0:H])
                else:
                    nc.vector.tensor_scalar_mul(o1[:, m], pAm[:, 0:H], 1.0)
            return o1

        for it in range(n_iter):
            last = it == n_iter - 1
            nst = new_tree()  # on the last iteration this is the final-pass tree
            # Software pipelining: channel c+1's H-conv is emitted BEFORE
            # channel c's W-conv, so the in-order PE always has a full
            # H-conv of independent work queued while W(c) waits on the o1
            # drain of channel c.
            emit_pmul(0)
            o1 = emit_H(0, last)
            for c in range(C):
                if c + 1 < C:
                    emit_pmul(c + 1)
                    o1_nxt = emit_H(c + 1, last)
                for m in range(NCH):
                    pBm = psB.tile([P, 512], F32, tag="pb")
                    if not last:
                        # Seed the bank with x0 (identity matmul); the W-conv
                        # accumulates on top so it ends holding x = x0 + s.
                        nc.tensor.matmul(
                            pBm[:, 0:W],
                            lhsT=ident[:],
                            rhs=x0c(c)[:, m],
                            start=True,
                            stop=False,
                        )
                    for j in range(NCW):
                        n0, n1 = (0, W) if (j == 0 and full_j0 and last) else _band(j, W)
                        nc.tensor.matmul(
                            pBm[:, n0:n1],
                            lhsT=o1[:, j, m * P : (m + 1) * P],
                            rhs=tw_sb[:, j, n0:n1],
                            start=(j == 0 and last),
                            stop=(j == NCW - 1),
                        )
                    if not last:
                        # e = exp(x) straight out of the bank.
                        nc.scalar.activation(
                            out=ebuf[:, c, m], in_=pBm[:, 0:W], func=AF.Exp
                        )
                    else:
                        nc.vector.tensor_add(
                            out=xbuf[:, c, m], in0=x0c(c)[:, m], in1=pBm[:, 0:W]
                        )
                o1 = o1_nxt if c + 1 < C else None
                if not last:
                    tree_step(nst, c, u_eng=nc.gpsimd if c in (3, 7) else nc.vector)
                else:
                    # fold the final log_softmax's exps/tree into this loop;
                    # ScalarE has no per-channel exps here.
                    if c in (3, 7, 11):
                        emit_exp_cg(xbuf[:, c - 3 : c + 1], c // 4)
                        tree_step(nst, c, with_recip=False, u_eng=nc.gpsimd)
                    elif c == 15:
                        emit_exp_cg(xbuf[:, 12:16], 3)
                        tree_step(nst, 13, with_recip=False)
                        tree_step(nst, 14, with_recip=False)
                        tree_step(nst, 15, with_recip=False)
                # splice in the previous sample's final subtract+DMA work
                if pending:
                    pending.pop(0)()
            st = nst
        return st  # the final-pass tree (holds S of log-softmax)

    def emit_final_tail(b, fst):
        """Ln + per-channel subtract/DMA closures for sample b."""
        # Dedicated tag: lball stays live while the next sample's softmax
        # reciprocal (tag "r") is being written.
        lball = smax.tile([P, NW], F32, tag="lb", name="lb")
        nc.scalar.activation(out=lball[:], in_=fst["S"][:], func=AF.Ln)
        lb_v = lball[:].rearrange("p (a b) -> p a b", a=NCH)
        pending = []
        for c in range(C):
            def mk(c=c):
                def go():
                    sout = stage.tile([P, NCH, W], F32, tag="sout")
                    eng = nc.gpsimd if c % 3 == 2 else nc.vector
                    eng.tensor_sub(out=sout[:], in0=xbuf[:, c], in1=lb_v)
                    nc.sync.dma_start(
                        out=out_d[b, c].rearrange("(j p) w -> p j w", p=P),
                        in_=sout[:],
                    )
                return go
            pending.append(mk())
        return pending

    pending = []
    for b in range(n_samples):
        th_sb, tw_sb = load_sample(b)
        st0 = new_tree()
        emit_prologue(st0)
        fst = conv_iters(b, th_sb, tw_sb, st0, pending)
        pending = emit_final_tail(b, fst)
    for fn in pending:
        fn()


def build_nc(n_samples=BPC, n_iter=N_ITER, full_j0=False):
    # Bacc (not plain Bass): its compile() pass legalizes multi-wait
    # instructions via InstEventSemaphore — walrus caps regular instructions
    # at ONE sync wait.
    nc = bacc.Bacc()
    x_in = nc.dram_tensor("x", [n_samples, C, H, W], BF16, kind="ExternalInput")
    th_in = nc.dram_tensor("th", [n_samples, H, H], BF16, kind="ExternalInput")
    tw_in = nc.dram_tensor("tw", [n_samples, W, W], BF16, kind="ExternalInput")
    id_in = nc.dram_tensor("ident", [P, P], BF16, kind="ExternalInput")
    out_d = nc.dram_tensor("out", [n_samples, C, H, W], F32, kind="ExternalOutput")
    with tile.TileContext(nc) as tc:
        with ExitStack() as ctx:
            _crf_kernel(
                ctx, tc, out_d, x_in, th_in, tw_in, id_in, n_samples, n_iter, full_j0
            )
    nc.finalize()
    return nc


def make_toeplitz(spacing, inv_theta, size, weight=1.0):
    """Banded symmetric Toeplitz matrix for the 1D 'same' correlation."""
    d = spacing * np.arange(-(FS // 2), FS // 2 + 1, dtype=np.float32)
    k = np.exp(-((d * inv_theta) ** 2) / 2.0).astype(np.float32)
    k[FS // 2] = 0.0
    t = np.zeros((size, size), dtype=np.float32)
    for tap in range(FS):
        off = tap - FS // 2  # out[h] += k[tap] * x[h + off]
        idx = np.arange(max(0, -off), min(size, size - off))
        t[idx + off, idx] = k[tap]
    return (t * weight).astype(np.float32)


def host_prep(x, spatial_spacings, smoothness_weight, inv_smoothness_theta):
    """Build per-sample Th (H-conv) and weight-scaled Tw (W-conv) matrices
    plus the bf16 copy of x; all conv-path operands ship as bf16."""
    import ml_dtypes

    w = float(np.asarray(smoothness_weight))
    th = np.stack(
        [
            make_toeplitz(float(spatial_spacings[b, 0]), float(inv_smoothness_theta[0]), H)
            for b in range(x.shape[0])
        ]
    ).astype(ml_dtypes.bfloat16)
    tw = np.stack(
        [
            make_toeplitz(
                float(spatial_spacings[b, 1]), float(inv_smoothness_theta[1]), W, weight=w
            )
            for b in range(x.shape[0])
        ]
    ).astype(ml_dtypes.bfloat16)
    xb = np.ascontiguousarray(x).astype(ml_dtypes.bfloat16)
    return xb, th, tw


def make_ident():
    import ml_dtypes

    return np.eye(P, dtype=np.float32).astype(ml_dtypes.bfloat16)


_NC_CACHE = {}


def kernel(x, spatial_spacings, smoothness_weight, inv_smoothness_theta):
    from concourse.bass_utils import run_bass_kernel_spmd

    x = np.ascontiguousarray(np.asarray(x), dtype=np.float32)
    spatial_spacings = np.asarray(spatial_spacings, dtype=np.float32)
    xb, th, tw = host_prep(x, spatial_spacings, smoothness_weight, inv_smoothness_theta)
    ident = make_ident()

    key = (BPC, N_ITER)
    if key not in _NC_CACHE:
        _NC_CACHE[key] = build_nc(BPC, N_ITER)
    nc = _NC_CACHE[key]

    core_ids = list(range(N_CORES))
    in_maps = []
    for i in core_ids:
        sl = slice(i * BPC, (i + 1) * BPC)
        in_maps.append({"x": xb[sl], "th": th[sl], "tw": tw[sl], "ident": ident})
    res = run_bass_kernel_spmd(nc, in_maps, core_ids)
    out = np.concatenate([res.results[i]["out"] for i in core_ids], axis=0)
    return out.astype(np.float32)


if __name__ == "__main__":
    rng = np.random.default_rng(0)
    x = rng.standard_normal((B, C, H, W), dtype=np.float32)
    out = kernel(
        x,
        np.ones((B, 2), np.float32),
        np.float32(1.0),
        np.ones((2,), np.float32),
    )
    print(out.shape, out.dtype)
